# revision 4
# baseline (speedup 1.0000x reference)
"""Trainium2 Bass kernel for CSSrcMapper (color-coded class map -> feature map).

Semantics (matches reference):
    d[b,c,h,w]  = floor(src[b,c,h,w] * 127.5 + 127.5)            (int color decode)
    match[b,k,h,w] = all_c(d[b,c,h,w] == colors[k,c])            (one-hot class)
    out[b,:,h,w] = sum_k match[b,k,h,w] * feats[k,:]             (feature scatter)

Strategy: data-parallel over 8 cores, shard = (batch, H-half).  The host
pre-replicates each source channel over a 32-row class group (f16 -- the
decode margin is 0.25, f16 error is <= ~0.03).  Per core and macro-tile:
 - one DMA loads the replicated [96, T] f16 block
 - ACT computes sq = (127.5*s + (127-color_k))^2 as bf16: squared distance
   to each class color per channel group (match ~1e-3, mismatch >= ~0.9)
 - a 0/1 selector matmul sums the three channel distances into match rows
   k and 32+k; sum < 0.25 is the one-hot class match (DVE is_lt from PSUM)
 - one K=128 matmul against stacked [hi;lo] bf16 feats performs the exact
   feature lookup (hi+lo split -> ~1e-5 relative error) per 128-channel chunk
 - PSUM -> SBUF copies alternate ACT/DVE, f16 DMA stores.
The kernel is HBM-write-bound: 64 MiB of f16 output per core (upcast to
f32 on the host; f16 rounding adds ~3e-4 norm relative error, well inside
the 2e-2 gate).
"""

from contextlib import ExitStack

import numpy as np
import ml_dtypes

import concourse.bass as bass
import concourse.mybir as mybir
import concourse.tile as tile
from concourse import bacc
from concourse.bass_utils import run_bass_kernel_spmd

B, H, W = 4, 256, 256
K = 19
FEAT = 1024
NCORES = 8
HSH = H // 2              # 128 rows per shard
NPIX = HSH * W            # 32768 pixels per core
TM = 4096                 # pixels per macro-tile
NCHUNK = FEAT // 128      # 8 output-channel chunks
SCALE = 127.5

f32 = mybir.dt.float32
f16 = mybir.dt.float16
bf16 = mybir.dt.bfloat16


def _build_nc(npix=NPIX, tm=TM):
    nmt = npix // tm
    nc = bacc.Bacc("TRN2", target_bir_lowering=False, debug=False)
    srcr = nc.dram_tensor("srcr", [57, npix], f16, kind="ExternalInput").ap()
    cols = nc.dram_tensor("cols", [57, 1], f32, kind="ExternalInput").ap()
    sel = nc.dram_tensor("sel", [57, 128], bf16, kind="ExternalInput").ap()
    fst = nc.dram_tensor("fst", [128, FEAT], bf16, kind="ExternalInput").ap()
    out = nc.dram_tensor("out", [FEAT, npix], f16, kind="ExternalOutput").ap()

    with tile.TileContext(nc) as tc, ExitStack() as ctx:
        const_p = ctx.enter_context(tc.tile_pool(name="const", bufs=1))
        sq_p = ctx.enter_context(tc.tile_pool(name="sqp", bufs=3))
        mps_p = ctx.enter_context(tc.tile_pool(name="mpsp", bufs=2, space="PSUM"))
        match_p = ctx.enter_context(tc.tile_pool(name="matchp", bufs=3))
        out_p = ctx.enter_context(tc.tile_pool(name="outp", bufs=4))
        psum_p = ctx.enter_context(tc.tile_pool(name="psum", bufs=3, space="PSUM"))

        colst = const_p.tile([57, 1], f32)
        nc.sync.dma_start(colst[:], cols[:])
        sel_sb = const_p.tile([57, 128], bf16)
        nc.sync.dma_start(sel_sb[:], sel[:])
        fst_sb = const_p.tile([128, FEAT], bf16)
        nc.sync.dma_start(fst_sb[:], fst[:])
        # whole-shard replicated source: loaded once during ramp, so the
        # steady state issues only output writes
        rc_all = const_p.tile([57, npix], f16)
        nc.sync.dma_start(rc_all[:], srcr[:])

        for m in range(nmt):
            msl = slice(m * tm, (m + 1) * tm)
            # squared distance to each class color per channel group
            sq = sq_p.tile([57, tm], bf16)
            nc.scalar.activation(
                sq[:], rc_all[:, msl], mybir.ActivationFunctionType.Square,
                bias=colst[:], scale=SCALE,
            )

            # sum the three channel distances into match rows k and 32+k;
            # sum < 0.25 <=> one-hot class match
            match = match_p.tile([128, tm], bf16)
            for n in range(tm // 512):
                nsl = slice(n * 512, (n + 1) * 512)
                mps = mps_p.tile(
                    [128, 512], f32, space="PSUM", name=f"mps_{m}_{n}", tag="mps"
                )
                nc.tensor.matmul(
                    mps[:], sel_sb[:], sq[:, nsl], start=True, stop=True
                )
                nc.vector.tensor_scalar(
                    match[:, nsl], mps[:], 0.25, None, mybir.AluOpType.is_lt
                )

            # K=128 stacked hi/lo lookup (rows 0..18 hi, 32..50 lo, rest 0);
            # full-array matmuls keep the PE activity monitor warm.
            for j in range(NCHUNK):
                jsl = slice(j * 128, (j + 1) * 128)
                ob = out_p.tile([128, tm], f16)
                for hh in range(tm // 1024):
                    ps = psum_p.tile([128, 1024], f32, space="PSUM")
                    for q in range(2):
                        nsl = slice(hh * 1024 + q * 512, hh * 1024 + q * 512 + 512)
                        qsl = slice(q * 512, (q + 1) * 512)
                        nc.tensor.matmul(
                            ps[:, qsl], fst_sb[:, jsl], match[:, nsl],
                            start=True, stop=True,
                        )
                    osl = slice(hh * 1024, (hh + 1) * 1024)
                    if (j * (tm // 1024) + hh) % 2 == 0:
                        nc.scalar.copy(ob[:, osl], ps[:])
                    else:
                        nc.vector.tensor_copy(ob[:, osl], ps[:])
                nc.sync.dma_start(out[jsl, msl], ob[:])
    nc.compile()
    return nc


_CACHE = {}


def _get_nc():
    if "nc" not in _CACHE:
        _CACHE["nc"] = _build_nc()
    return _CACHE["nc"]


def _host_prep(src, colors, feats):
    src = np.asarray(src, dtype=np.float32)
    colors = np.asarray(colors, dtype=np.int32)
    feats = np.asarray(feats, dtype=np.float32)

    colstack = np.empty((57, 1), dtype=np.float32)
    for c in range(3):
        colstack[c * K:(c + 1) * K, 0] = 127.0 - colors[:, c].astype(np.float32)
    selmat = np.zeros((57, 128), dtype=ml_dtypes.bfloat16)
    for c in range(3):
        for k in range(K):
            selmat[c * K + k, k] = 1
            selmat[c * K + k, 32 + k] = 1
    fhi = feats.astype(ml_dtypes.bfloat16)
    flo = (feats - fhi.astype(np.float32)).astype(ml_dtypes.bfloat16)
    fstack = np.zeros((128, FEAT), dtype=ml_dtypes.bfloat16)
    fstack[0:K] = fhi
    fstack[32:32 + K] = flo

    in_maps = []
    for core in range(NCORES):
        b, half = divmod(core, 2)
        shard = np.ascontiguousarray(
            src[b, :, half * HSH:(half + 1) * HSH, :]
        ).reshape(3, NPIX).astype(np.float16)
        shard_rep = np.repeat(shard, K, axis=0)   # [57, NPIX], channel-grouped
        in_maps.append(
            {"srcr": shard_rep, "cols": colstack, "sel": selmat, "fst": fstack}
        )
    return in_maps


def _assemble(results):
    full = np.empty((B, FEAT, H, W), dtype=np.float32)
    for core in range(NCORES):
        b, half = divmod(core, 2)
        full[b, :, half * HSH:(half + 1) * HSH, :] = results[core]["out"].reshape(
            FEAT, HSH, W
        )
    return full


def kernel(src, colors, feats):
    nc = _get_nc()
    in_maps = _host_prep(src, colors, feats)
    res = run_bass_kernel_spmd(nc, in_maps, list(range(NCORES)))
    return _assemble(res.results)

